# revision 18
# baseline (speedup 1.0000x reference)
"""Trainium2 Bass kernel for per-8x8-block DCT -> quantize(round) -> IDCT.

Math (per 8x8 spatial block B of x, with orthonormal DCT matrix D):
    X  = D @ B @ D.T          (2D DCT)
    Xq = round(X / q) * q     (quantize / dequantize)
    B' = D.T @ Xq @ D         (2D inverse DCT)

Implementation strategy (per core, data-parallel over the batch axis N):
  - DMA x in as [partition=c(128), free=(16 rows x 112 cols)] chunks --
    fully contiguous 7KB-per-partition runs, full HBM rate.
  - GpSimd gather-copy to block-contiguous layout f' = wb*64 + j*8 + k.
  - PE transpose-mode (T1) moves each pair of 8x8 blocks into "Kron
    layout" [partition=(s,j,k) in [0,128), free=c], where s indexes the
    block pair, (j,k) the position inside the block.
  - One 128x128 stationary matmul MM1 = blockdiag(G, G), G = (D kron D),
    computes the full 2D DCT of both blocks in a single fp32 pass.
  - Quantize in two VectorE ops: t = X*(1/q) + 1.5*2^23 (the fp32 add
    itself rounds to integer with round-half-even, exactly matching
    jnp.round), then subtract the magic constant and cast to fp16 (both
    exact -- the rounded values are small integers).  1/q is a
    per-partition scalar in this layout.
  - MM2 fused with the output transpose: a regular fp16 matmul with the
    *rounded data as the stationary operand* computes
    out[c, (j,s,k)] = sum_(s,il) rt2[(s,il), c] * W2[(s,il), (j,s,k)]
    where W2 = blockdiag(G.T diag(q)) with columns permuted to (j,s,k)
    order, so the result lands directly in [partition=c] orientation.
  - C3: VectorE tensor_sub writes PSUM -> SBUF into the natural (h,w)
    free-axis layout (3-dim scatter AP) while subtracting the constant
    1536-offset correction vector (host-precomputed, replicated tile).
  - DMA out, fully contiguous again.
"""

import numpy as np

N_FULL, C, H, W = 32, 128, 112, 112
N_CORES = 8
N_SHARD = N_FULL // N_CORES  # 4
BLK = 8
HB = H // BLK  # 14
WB = W // BLK  # 14
ROWCHUNK = BLK * W  # 896 floats per (c, hb) chunk
CMAGIC = float(np.float32(1.5 * 2**23))  # fp32 round-to-int magic

LAST_EXEC_NS = None
LAST_PROFILE = None


def _dct_mat():
    k = np.arange(BLK, dtype=np.float64)[:, None]
    m = np.arange(BLK, dtype=np.float64)[None, :]
    D = np.cos(np.pi * (2.0 * m + 1.0) * k / (2.0 * BLK))
    D[0, :] *= np.sqrt(1.0 / BLK)
    D[1:, :] *= np.sqrt(2.0 / BLK)
    return D.astype(np.float32)


def _build_consts(q_table: np.ndarray):
    D = _dct_mat()
    # G[(i,l),(j,k)] = D[i,j] * D[l,k]; forward DCT: Xvec = G @ xvec
    G = np.einsum("ij,lk->iljk", D, D).reshape(64, 64).astype(np.float32)
    # MM1 stationary lhsT[(s,jk),(s,il)] = G[(il),(jk)]  (blockdiag over s)
    G1 = np.zeros((128, 128), np.float32)
    G1[:64, :64] = G.T
    G1[64:, 64:] = G.T
    qflat = q_table.reshape(64).astype(np.float32)
    qinv = np.concatenate([1.0 / qflat, 1.0 / qflat]).reshape(128, 1)
    # W2[(s,il),(s,jk)] = G[(il),(jk)] * q[(il)], blockdiag over s, with
    # columns permuted from (s,j,k) to (j,s,k) order for contiguous scatter
    W2s = (G * qflat[:, None]).astype(np.float32)
    W2 = np.zeros((128, 128), np.float32)
    W2[:64, :64] = W2s
    W2[64:, 64:] = W2s
    perm = np.zeros(128, np.int64)  # m = j*16 + s*8 + k  <-  col s*64+j*8+k
    for j in range(8):
        for s in range(2):
            for k in range(8):
                perm[j * 16 + s * 8 + k] = s * 64 + j * 8 + k
    W2p = W2[:, perm].astype(np.float16)
    I32 = np.eye(128, dtype=np.float32)
    return G1, W2p, qinv.astype(np.float32), I32


def _build_program():
    import concourse.mybir as mybir
    from concourse import bacc
    from concourse.tile import TileContext

    fp32 = mybir.dt.float32
    fp16 = mybir.dt.float16

    nc = bacc.Bacc()
    xs = nc.declare_dram_parameter("x", [N_SHARD, C, H, W], fp32, isOutput=False)
    g1 = nc.declare_dram_parameter("g1", [128, 128], fp32, isOutput=False)
    w2 = nc.declare_dram_parameter("w2", [128, 128], fp16, isOutput=False)
    qinv = nc.declare_dram_parameter("qinv", [128, 1], fp32, isOutput=False)
    id32 = nc.declare_dram_parameter("id32", [128, 128], fp32, isOutput=False)
    ys = nc.declare_dram_parameter("y", [N_SHARD, C, H, W], fp32, isOutput=True)

    halves = [(0, 4), (4, 7)]  # wbp groups: 512 + 384 columns
    HBP = HB // 2  # 7 hb-pairs per image
    PAIRCHUNK = 2 * ROWCHUNK

    with TileContext(nc) as tc:
        with (
            tc.tile_pool(name="const", bufs=1) as cpool,
            tc.tile_pool(name="io", bufs=3) as iopool,
            tc.tile_pool(name="work", bufs=2) as wpool,
            tc.tile_pool(name="psum", bufs=2, space="PSUM") as ppool,
        ):
            ident32 = cpool.tile([128, 128], fp32, tag="id32")
            nc.sync.dma_start(out=ident32[:, :], in_=id32[:, :])
            g1_t = cpool.tile([128, 128], fp32, tag="g1")
            nc.sync.dma_start(out=g1_t[:, :], in_=g1[:, :])
            w2_t = cpool.tile([128, 128], fp16, tag="w2")
            nc.sync.dma_start(out=w2_t[:, :], in_=w2[:, :])
            qinv_t = cpool.tile([128, 1], fp32, tag="qinv")
            nc.sync.dma_start(out=qinv_t[:, :], in_=qinv[:, :])

            for n in range(N_SHARD):
                x_flat = xs[n, :, :, :].rearrange("c h w -> c (h w)")
                y_flat = ys[n, :, :, :].rearrange("c h w -> c (h w)")
                for hbp in range(HBP):
                    xt = iopool.tile([128, PAIRCHUNK], fp32, tag="xt")
                    nc.sync.dma_start(
                        out=xt[:, :],
                        in_=x_flat[:, hbp * PAIRCHUNK : (hbp + 1) * PAIRCHUNK],
                    )
                    yt = iopool.tile([128, PAIRCHUNK], fp32, tag="yt")
                    for hl in range(2):
                        base = hl * ROWCHUNK
                        xt_g = xt[:, base : base + ROWCHUNK].rearrange(
                            "c (j wb k) -> c wb j k", j=8, wb=WB, k=8
                        )
                        # out free layout: f = wbp*16 + j... -> (wbp, j, sk)
                        yt_g = yt[:, base : base + ROWCHUNK].rearrange(
                            "c (j wp sk) -> c wp j sk", j=8, wp=WB // 2, sk=16
                        )
                        # gather to block-contiguous f' = wb*64 + j*8 + k
                        xg = wpool.tile([128, ROWCHUNK], fp32, tag="xg")
                        xg_g = xg[:, :].rearrange(
                            "c (wb j k) -> c wb j k", wb=WB, j=8, k=8
                        )
                        nc.gpsimd.tensor_copy(xg_g, xt_g)

                        for w0, w1 in halves:
                            nw = w1 - w0
                            ncols = nw * 128
                            # --- T1: blocks -> Kron layout [(s,j,k), c] ---
                            t1p = ppool.tile([128, 512], fp32, tag="t1")
                            for ii, wbp in enumerate(range(w0, w1)):
                                nc.tensor.transpose(
                                    t1p[:, ii * 128 : (ii + 1) * 128],
                                    xg[:, wbp * 128 : (wbp + 1) * 128],
                                    ident32[:, :],
                                )
                            kt = wpool.tile([128, 512], fp32, tag="kt")
                            nc.scalar.copy(kt[:, :ncols], t1p[:, :ncols])
                            # --- MM1: forward 2D DCT (fp32) ---
                            mm1p = ppool.tile([128, 512], fp32, tag="mm1")
                            nc.tensor.matmul(
                                mm1p[:, :ncols], g1_t[:, :], kt[:, :ncols],
                                start=True, stop=True,
                            )
                            # --- quantize: X/q + 1.5*2^23 rounds via add ---
                            rt2a = wpool.tile([128, 512], fp32, tag="rt2a")
                            nc.vector.tensor_scalar(
                                out=rt2a[:, :ncols], in0=mm1p[:, :ncols],
                                scalar1=qinv_t[:, :], scalar2=CMAGIC,
                                op0=mybir.AluOpType.mult,
                                op1=mybir.AluOpType.add,
                            )
                            # remove the magic; cast to fp16 (both exact)
                            rt2 = wpool.tile([128, 512], fp16, tag="rt2")
                            nc.vector.tensor_scalar_sub(
                                rt2[:, :ncols], rt2a[:, :ncols], CMAGIC
                            )
                            # --- MM2 (+fused transpose): data stationary ---
                            mm2p = ppool.tile([128, 512], fp32, tag="mm2")
                            for ii, wbp in enumerate(range(w0, w1)):
                                nc.tensor.matmul(
                                    mm2p[:, ii * 128 : (ii + 1) * 128],
                                    rt2[:, ii * 128 : (ii + 1) * 128],
                                    w2_t[:, :],
                                    start=True, stop=True,
                                )
                            # --- C3: scatter back to natural (h,w) layout ---
                            nc.scalar.copy(
                                yt_g[:, w0:w1, :, :],
                                mm2p[:, :ncols].rearrange(
                                    "c (wp j sk) -> c wp j sk", wp=nw, j=8, sk=16
                                ),
                            )
                    nc.sync.dma_start(
                        out=y_flat[:, hbp * PAIRCHUNK : (hbp + 1) * PAIRCHUNK],
                        in_=yt[:, :],
                    )
    return nc


_PROGRAM = None


def kernel(x: np.ndarray, q_table: np.ndarray) -> np.ndarray:
    global _PROGRAM, LAST_EXEC_NS, LAST_PROFILE
    from concourse.bass_utils import run_bass_kernel_spmd

    x = np.ascontiguousarray(np.asarray(x, dtype=np.float32))
    q_table = np.asarray(q_table, dtype=np.float32)
    assert x.shape == (N_FULL, C, H, W), x.shape

    G1, W2p, qinv, I32 = _build_consts(q_table)
    if _PROGRAM is None:
        nc = _build_program()
        nc.finalize()
        _PROGRAM = nc
    nc = _PROGRAM

    core_ids = list(range(N_CORES))
    in_maps = []
    for i in core_ids:
        shard = np.ascontiguousarray(x[i * N_SHARD : (i + 1) * N_SHARD])
        in_maps.append(
            {"x": shard, "g1": G1, "w2": W2p, "qinv": qinv, "id32": I32}
        )

    res = run_bass_kernel_spmd(nc, in_maps, core_ids)
    LAST_EXEC_NS = res.exec_time_ns
    LAST_PROFILE = res.profile_json
    out = np.concatenate([np.asarray(res.results[i]["y"]) for i in core_ids], axis=0)
    return out.astype(np.float32)


# revision 22
# speedup vs baseline: 1.4404x; 1.4404x over previous
"""Trainium2 Bass kernel for per-8x8-block DCT -> quantize(round) -> IDCT.

Math (per 8x8 spatial block B of x, with orthonormal DCT matrix D):
    X  = D @ B @ D.T          (2D DCT)
    Xq = round(X / q) * q     (quantize / dequantize)
    B' = D.T @ Xq @ D         (2D inverse DCT)

Implementation strategy (per core, data-parallel over the batch axis N):
  - DMA x in as [partition=c(128), free=(16 rows x 112 cols)] chunks --
    fully contiguous 7KB-per-partition runs, full HBM rate.
  - GpSimd gather-copy to block-contiguous layout f' = wb*64 + j*8 + k.
  - PE transpose-mode (T1) moves each pair of 8x8 blocks into "Kron
    layout" [partition=(s,j,k) in [0,128), free=c], where s indexes the
    block pair, (j,k) the position inside the block.
  - One 128x128 stationary matmul MM1 = blockdiag(G, G), G = (D kron D),
    computes the full 2D DCT of both blocks in a single fp32 pass.
  - Quantize in two VectorE ops: t = X*(1/q) + 1.5*2^23 (the fp32 add
    itself rounds to integer with round-half-even, exactly matching
    jnp.round), then subtract the magic constant and cast to fp16 (both
    exact -- the rounded values are small integers).  1/q is a
    per-partition scalar in this layout.
  - MM2 fused with the output transpose: a regular fp16 matmul with the
    *rounded data as the stationary operand* computes
    out[c, (j,s,k)] = sum_(s,il) rt2[(s,il), c] * W2[(s,il), (j,s,k)]
    where W2 = blockdiag(G.T diag(q)) with columns permuted to (j,s,k)
    order, so the result lands directly in [partition=c] orientation.
  - C3: VectorE tensor_sub writes PSUM -> SBUF into the natural (h,w)
    free-axis layout (3-dim scatter AP) while subtracting the constant
    1536-offset correction vector (host-precomputed, replicated tile).
  - DMA out, fully contiguous again.
"""

import numpy as np

N_FULL, C, H, W = 32, 128, 112, 112
N_CORES = 8
N_SHARD = N_FULL // N_CORES  # 4
BLK = 8
HB = H // BLK  # 14
WB = W // BLK  # 14
ROWCHUNK = BLK * W  # 896 floats per (c, hb) chunk
CMAGIC = float(np.float32(1.5 * 2**23))  # fp32 round-to-int magic

LAST_EXEC_NS = None
LAST_PROFILE = None


def _dct_mat():
    k = np.arange(BLK, dtype=np.float64)[:, None]
    m = np.arange(BLK, dtype=np.float64)[None, :]
    D = np.cos(np.pi * (2.0 * m + 1.0) * k / (2.0 * BLK))
    D[0, :] *= np.sqrt(1.0 / BLK)
    D[1:, :] *= np.sqrt(2.0 / BLK)
    return D.astype(np.float32)


def _build_consts(q_table: np.ndarray):
    D = _dct_mat()
    # G[(i,l),(j,k)] = D[i,j] * D[l,k]; forward DCT: Xvec = G @ xvec
    G = np.einsum("ij,lk->iljk", D, D).reshape(64, 64).astype(np.float32)
    # MM1 stationary lhsT[(s,jk),(s,il)] = G[(il),(jk)] / q[(il)]
    # (blockdiag over s; the quantizer divide is folded into the columns)
    qflat = q_table.reshape(64).astype(np.float32)
    Gq = (G.T / qflat[None, :]).astype(np.float32)
    G1 = np.zeros((128, 128), np.float32)
    G1[:64, :64] = Gq
    G1[64:, 64:] = Gq
    # W2[(s,il),(s,jk)] = G[(il),(jk)] * q[(il)], blockdiag over s, with
    # columns permuted from (s,j,k) to (j,s,k) order for contiguous scatter
    W2s = (G * qflat[:, None]).astype(np.float32)
    W2 = np.zeros((128, 128), np.float32)
    W2[:64, :64] = W2s
    W2[64:, 64:] = W2s
    perm = np.zeros(128, np.int64)  # m = j*16 + s*8 + k  <-  col s*64+j*8+k
    for j in range(8):
        for s in range(2):
            for k in range(8):
                perm[j * 16 + s * 8 + k] = s * 64 + j * 8 + k
    W2p = W2[:, perm].astype(np.float16)
    I32 = np.eye(128, dtype=np.float32)
    return G1, W2p, I32


def _build_program(mm_bufs=2, t1_bufs=3, mm2_bufs=None, c3_split=True, io_bufs=3,
                   work_bufs=2, xg_split=True):
    import concourse.mybir as mybir
    from concourse import bacc
    from concourse.tile import TileContext

    fp32 = mybir.dt.float32
    fp16 = mybir.dt.float16

    nc = bacc.Bacc()
    xs = nc.declare_dram_parameter("x", [N_SHARD, C, H, W], fp32, isOutput=False)
    g1 = nc.declare_dram_parameter("g1", [128, 128], fp32, isOutput=False)
    w2 = nc.declare_dram_parameter("w2", [128, 128], fp16, isOutput=False)
    id32 = nc.declare_dram_parameter("id32", [128, 128], fp32, isOutput=False)
    ys = nc.declare_dram_parameter("y", [N_SHARD, C, H, W], fp32, isOutput=True)

    halves = [(0, 4), (4, 7)]  # wbp groups: 512 + 384 columns
    HBP = HB // 2  # 7 hb-pairs per image
    PAIRCHUNK = 2 * ROWCHUNK

    with TileContext(nc) as tc:
        with (
            tc.tile_pool(name="const", bufs=1) as cpool,
            tc.tile_pool(name="io", bufs=io_bufs) as iopool,
            tc.tile_pool(name="work", bufs=work_bufs) as wpool,
            tc.tile_pool(name="psum", bufs=t1_bufs, space="PSUM") as ppool,
            tc.tile_pool(name="psum3", bufs=mm_bufs, space="PSUM") as ppool3,
            tc.tile_pool(name="psum4", bufs=(mm2_bufs or mm_bufs), space="PSUM") as ppool4,
        ):
            ident32 = cpool.tile([128, 128], fp32, tag="id32")
            nc.sync.dma_start(out=ident32[:, :], in_=id32[:, :])
            g1_t = cpool.tile([128, 128], fp32, tag="g1")
            nc.sync.dma_start(out=g1_t[:, :], in_=g1[:, :])
            w2_t = cpool.tile([128, 128], fp16, tag="w2")
            nc.sync.dma_start(out=w2_t[:, :], in_=w2[:, :])

            for n in range(N_SHARD):
                x_flat = xs[n, :, :, :].rearrange("c h w -> c (h w)")
                y_flat = ys[n, :, :, :].rearrange("c h w -> c (h w)")
                for hbp in range(HBP):
                    xt = iopool.tile([128, PAIRCHUNK], fp32, tag="xt")
                    nc.sync.dma_start(
                        out=xt[:, :],
                        in_=x_flat[:, hbp * PAIRCHUNK : (hbp + 1) * PAIRCHUNK],
                    )
                    yt = iopool.tile([128, PAIRCHUNK], fp32, tag="yt")
                    for hl in range(2):
                        base = hl * ROWCHUNK
                        xt_g = xt[:, base : base + ROWCHUNK].rearrange(
                            "c (j wb k) -> c wb j k", j=8, wb=WB, k=8
                        )
                        # out free layout: f = wbp*16 + j... -> (wbp, j, sk)
                        yt_g = yt[:, base : base + ROWCHUNK].rearrange(
                            "c (j wp sk) -> c wp j sk", j=8, wp=WB // 2, sk=16
                        )
                        # gather to block-contiguous f' = wb*64 + j*8 + k
                        xg = wpool.tile([128, ROWCHUNK], fp32, tag="xg")
                        xg_g = xg[:, :].rearrange(
                            "c (wb j k) -> c wb j k", wb=WB, j=8, k=8
                        )
                        if xg_split:
                            nc.gpsimd.tensor_copy(
                                xg_g[:, :8, :, :], xt_g[:, :8, :, :]
                            )
                            nc.gpsimd.tensor_copy(
                                xg_g[:, 8:, :, :], xt_g[:, 8:, :, :]
                            )
                        else:
                            nc.gpsimd.tensor_copy(xg_g, xt_g)

                        for w0, w1 in halves:
                            nw = w1 - w0
                            ncols = nw * 128
                            # --- T1: blocks -> Kron layout [(s,j,k), c] ---
                            t1p = ppool.tile([128, 512], fp32, tag="t1")
                            for ii, wbp in enumerate(range(w0, w1)):
                                nc.tensor.transpose(
                                    t1p[:, ii * 128 : (ii + 1) * 128],
                                    xg[:, wbp * 128 : (wbp + 1) * 128],
                                    ident32[:, :],
                                )
                            kt = wpool.tile([128, 512], fp32, tag="kt")
                            nc.scalar.copy(kt[:, :ncols], t1p[:, :ncols])
                            # --- MM1: forward 2D DCT (fp32) ---
                            mm1p = ppool3.tile([128, 512], fp32, tag="mm1")
                            nc.tensor.matmul(
                                mm1p[:, :ncols], g1_t[:, :], kt[:, :ncols],
                                start=True, stop=True,
                            )
                            # --- quantize: (X/q + 1.5*2^23) - 1.5*2^23 ---
                            # the fp32 add rounds to integer (RNE, matching
                            # jnp.round); the sub and fp16 cast are exact
                            rt2 = wpool.tile([128, 512], fp16, tag="rt2")
                            nc.vector.tensor_scalar(
                                out=rt2[:, :ncols], in0=mm1p[:, :ncols],
                                scalar1=CMAGIC, scalar2=CMAGIC,
                                op0=mybir.AluOpType.add,
                                op1=mybir.AluOpType.subtract,
                            )
                            # --- MM2 (+fused transpose): data stationary ---
                            mm2p = ppool4.tile([128, 512], fp32, tag="mm2")
                            for ii, wbp in enumerate(range(w0, w1)):
                                nc.tensor.matmul(
                                    mm2p[:, ii * 128 : (ii + 1) * 128],
                                    rt2[:, ii * 128 : (ii + 1) * 128],
                                    w2_t[:, :],
                                    start=True, stop=True,
                                )
                            # --- C3: scatter back to natural (h,w) layout ---
                            c3_src = mm2p[:, :ncols].rearrange(
                                "c (wp j sk) -> c wp j sk", wp=nw, j=8, sk=16
                            )
                            if w0 == 0 or not c3_split:
                                nc.scalar.copy(yt_g[:, w0:w1, :, :], c3_src)
                            else:
                                nc.vector.tensor_copy(yt_g[:, w0:w1, :, :], c3_src)
                    nc.sync.dma_start(
                        out=y_flat[:, hbp * PAIRCHUNK : (hbp + 1) * PAIRCHUNK],
                        in_=yt[:, :],
                    )
    return nc


_PROGRAM = None


def kernel(x: np.ndarray, q_table: np.ndarray) -> np.ndarray:
    global _PROGRAM, LAST_EXEC_NS, LAST_PROFILE
    from concourse.bass_utils import run_bass_kernel_spmd

    x = np.ascontiguousarray(np.asarray(x, dtype=np.float32))
    q_table = np.asarray(q_table, dtype=np.float32)
    assert x.shape == (N_FULL, C, H, W), x.shape

    G1, W2p, I32 = _build_consts(q_table)
    if _PROGRAM is None:
        nc = _build_program()
        nc.finalize()
        _PROGRAM = nc
    nc = _PROGRAM

    core_ids = list(range(N_CORES))
    in_maps = []
    for i in core_ids:
        shard = np.ascontiguousarray(x[i * N_SHARD : (i + 1) * N_SHARD])
        in_maps.append({"x": shard, "g1": G1, "w2": W2p, "id32": I32})

    res = run_bass_kernel_spmd(nc, in_maps, core_ids)
    LAST_EXEC_NS = res.exec_time_ns
    LAST_PROFILE = res.profile_json
    out = np.concatenate([np.asarray(res.results[i]["y"]) for i in core_ids], axis=0)
    return out.astype(np.float32)


# revision 25
# speedup vs baseline: 33.2410x; 23.0776x over previous
"""Trainium2 Bass kernel for per-8x8-block DCT -> quantize(round) -> IDCT.

Math (per 8x8 spatial block B of x, with orthonormal DCT matrix D):
    X  = D @ B @ D.T          (2D DCT)
    Xq = round(X / q) * q     (quantize / dequantize)
    B' = D.T @ Xq @ D         (2D inverse DCT)

Implementation strategy (per core, data-parallel over the batch axis N):
  - DMA x in as [partition=c(128), free=(16 rows x 112 cols)] chunks --
    fully contiguous 7KB-per-partition runs, full HBM rate.
  - GpSimd gather-copy to block-contiguous layout f' = wb*64 + j*8 + k.
  - PE transpose-mode (T1) moves each pair of 8x8 blocks into "Kron
    layout" [partition=(s,j,k) in [0,128), free=c], where s indexes the
    block pair, (j,k) the position inside the block.
  - One 128x128 stationary matmul MM1 = blockdiag(G, G), G = (D kron D),
    computes the full 2D DCT of both blocks in a single fp32 pass.
  - Quantize in two VectorE ops: t = X*(1/q) + 1.5*2^23 (the fp32 add
    itself rounds to integer with round-half-even, exactly matching
    jnp.round), then subtract the magic constant and cast to fp16 (both
    exact -- the rounded values are small integers).  1/q is a
    per-partition scalar in this layout.
  - MM2 fused with the output transpose: a regular fp16 matmul with the
    *rounded data as the stationary operand* computes
    out[c, (j,s,k)] = sum_(s,il) rt2[(s,il), c] * W2[(s,il), (j,s,k)]
    where W2 = blockdiag(G.T diag(q)) with columns permuted to (j,s,k)
    order, so the result lands directly in [partition=c] orientation.
  - C3: VectorE tensor_sub writes PSUM -> SBUF into the natural (h,w)
    free-axis layout (3-dim scatter AP) while subtracting the constant
    1536-offset correction vector (host-precomputed, replicated tile).
  - DMA out, fully contiguous again.
"""

import numpy as np

N_FULL, C, H, W = 32, 128, 112, 112
N_CORES = 8
N_SHARD = N_FULL // N_CORES  # 4
BLK = 8
HB = H // BLK  # 14
WB = W // BLK  # 14
ROWCHUNK = BLK * W  # 896 floats per (c, hb) chunk
CMAGIC = float(np.float32(1.5 * 2**23))  # fp32 round-to-int magic

LAST_EXEC_NS = None
LAST_PROFILE = None


def _dct_mat():
    k = np.arange(BLK, dtype=np.float64)[:, None]
    m = np.arange(BLK, dtype=np.float64)[None, :]
    D = np.cos(np.pi * (2.0 * m + 1.0) * k / (2.0 * BLK))
    D[0, :] *= np.sqrt(1.0 / BLK)
    D[1:, :] *= np.sqrt(2.0 / BLK)
    return D.astype(np.float32)


def _build_consts(q_table: np.ndarray):
    D = _dct_mat()
    # G[(i,l),(j,k)] = D[i,j] * D[l,k]; forward DCT: Xvec = G @ xvec
    G = np.einsum("ij,lk->iljk", D, D).reshape(64, 64).astype(np.float32)
    # MM1 stationary lhsT[(s,jk),(s,il)] = G[(il),(jk)] / q[(il)]
    # (blockdiag over s; the quantizer divide is folded into the columns)
    qflat = q_table.reshape(64).astype(np.float32)
    Gq = (G.T / qflat[None, :]).astype(np.float32)
    G1 = np.zeros((128, 128), np.float32)
    G1[:64, :64] = Gq
    G1[64:, 64:] = Gq
    # W2[(s,il),(s,jk)] = G[(il),(jk)] * q[(il)], blockdiag over s, with
    # columns permuted from (s,j,k) to (j,s,k) order for contiguous scatter
    W2s = (G * qflat[:, None]).astype(np.float32)
    W2 = np.zeros((128, 128), np.float32)
    W2[:64, :64] = W2s
    W2[64:, 64:] = W2s
    perm = np.zeros(128, np.int64)  # m = j*16 + s*8 + k  <-  col s*64+j*8+k
    for j in range(8):
        for s in range(2):
            for k in range(8):
                perm[j * 16 + s * 8 + k] = s * 64 + j * 8 + k
    W2p = W2[:, perm].astype(np.float16)
    I32 = np.eye(128, dtype=np.float32)
    return G1, W2p, I32


def _build_program(mm_bufs=2, t1_bufs=3, mm2_bufs=None, c3_split=True, io_bufs=3,
                   work_bufs=2, xg_split=True, repeat=1, warmers=True,
                   t1_regular=False):
    import concourse.mybir as mybir
    from concourse import bacc
    from concourse.tile import TileContext

    fp32 = mybir.dt.float32
    fp16 = mybir.dt.float16

    nc = bacc.Bacc()
    xs = nc.declare_dram_parameter("x", [N_SHARD, C, H, W], fp32, isOutput=False)
    g1 = nc.declare_dram_parameter("g1", [128, 128], fp32, isOutput=False)
    w2 = nc.declare_dram_parameter("w2", [128, 128], fp16, isOutput=False)
    id32 = nc.declare_dram_parameter("id32", [128, 128], fp32, isOutput=False)
    ys = nc.declare_dram_parameter("y", [N_SHARD, C, H, W], fp32, isOutput=True)

    halves = [(0, 4), (4, 7)]  # wbp groups: 512 + 384 columns
    HBP = HB // 2  # 7 hb-pairs per image
    PAIRCHUNK = 2 * ROWCHUNK

    with TileContext(nc) as tc:
        with (
            tc.tile_pool(name="const", bufs=1) as cpool,
            tc.tile_pool(name="io", bufs=io_bufs) as iopool,
            tc.tile_pool(name="work", bufs=work_bufs) as wpool,
            tc.tile_pool(name="psum", bufs=t1_bufs, space="PSUM") as ppool,
            tc.tile_pool(name="psum3", bufs=mm_bufs, space="PSUM") as ppool3,
            tc.tile_pool(name="psum4", bufs=(mm2_bufs or mm_bufs), space="PSUM") as ppool4,
            tc.tile_pool(name="psumw", bufs=1, space="PSUM") as ppoolw,
        ):
            ident32 = cpool.tile([128, 128], fp32, tag="id32")
            nc.sync.dma_start(out=ident32[:, :], in_=id32[:, :])
            g1_t = cpool.tile([128, 128], fp32, tag="g1")
            nc.sync.dma_start(out=g1_t[:, :], in_=g1[:, :])
            w2_t = cpool.tile([128, 128], fp16, tag="w2")
            nc.sync.dma_start(out=w2_t[:, :], in_=w2[:, :])

            for rep in range(repeat):
              for n in range(N_SHARD):
                x_flat = xs[n, :, :, :].rearrange("c h w -> c (h w)")
                y_flat = ys[n, :, :, :].rearrange("c h w -> c (h w)")
                for hbp in range(HBP):
                    xt = iopool.tile([128, PAIRCHUNK], fp32, tag="xt")
                    nc.sync.dma_start(
                        out=xt[:, :],
                        in_=x_flat[:, hbp * PAIRCHUNK : (hbp + 1) * PAIRCHUNK],
                    )
                    yt = iopool.tile([128, PAIRCHUNK], fp32, tag="yt")
                    for hl in range(2):
                        base = hl * ROWCHUNK
                        xt_g = xt[:, base : base + ROWCHUNK].rearrange(
                            "c (j wb k) -> c wb j k", j=8, wb=WB, k=8
                        )
                        # out free layout: f = wbp*16 + j... -> (wbp, j, sk)
                        yt_g = yt[:, base : base + ROWCHUNK].rearrange(
                            "c (j wp sk) -> c wp j sk", j=8, wp=WB // 2, sk=16
                        )
                        # gather to block-contiguous f' = wb*64 + j*8 + k
                        xg = wpool.tile([128, ROWCHUNK], fp32, tag="xg")
                        xg_g = xg[:, :].rearrange(
                            "c (wb j k) -> c wb j k", wb=WB, j=8, k=8
                        )
                        if xg_split:
                            nc.gpsimd.tensor_copy(
                                xg_g[:, :8, :, :], xt_g[:, :8, :, :]
                            )
                            nc.gpsimd.tensor_copy(
                                xg_g[:, 8:, :, :], xt_g[:, 8:, :, :]
                            )
                        else:
                            nc.gpsimd.tensor_copy(xg_g, xt_g)

                        for w0, w1 in halves:
                            nw = w1 - w0
                            ncols = nw * 128
                            # --- T1: blocks -> Kron layout [(s,j,k), c] ---
                            t1p = ppool.tile([128, 512], fp32, tag="t1")
                            for ii, wbp in enumerate(range(w0, w1)):
                                if t1_regular:
                                    nc.tensor.matmul(
                                        t1p[:, ii * 128 : (ii + 1) * 128],
                                        xg[:, wbp * 128 : (wbp + 1) * 128],
                                        ident32[:, :],
                                        start=True, stop=True,
                                    )
                                else:
                                    nc.tensor.transpose(
                                        t1p[:, ii * 128 : (ii + 1) * 128],
                                        xg[:, wbp * 128 : (wbp + 1) * 128],
                                        ident32[:, :],
                                    )
                            kt = wpool.tile([128, 512], fp32, tag="kt")
                            nc.scalar.copy(kt[:, :ncols], t1p[:, :ncols])
                            if warmers:
                                # tiny regular matmul anchored to this
                                # iteration: transpose-mode issue does not
                                # count as PE activity for the HAM clock
                                # gate, so without these the PE idles back
                                # to 1.2 GHz.  ~30ns each keeps K=8/8.
                                wp = ppoolw.tile([128, 64], fp32, tag="warm")
                                nc.tensor.matmul(
                                    wp[:, :], g1_t[:, :], kt[:, :64],
                                    start=True, stop=True,
                                )
                            # --- MM1: forward 2D DCT (fp32) ---
                            mm1p = ppool3.tile([128, 512], fp32, tag="mm1")
                            nc.tensor.matmul(
                                mm1p[:, :ncols], g1_t[:, :], kt[:, :ncols],
                                start=True, stop=True,
                            )
                            # --- quantize: (X/q + 1.5*2^23) - 1.5*2^23 ---
                            # the fp32 add rounds to integer (RNE, matching
                            # jnp.round); the sub and fp16 cast are exact
                            rt2 = wpool.tile([128, 512], fp16, tag="rt2")
                            nc.vector.tensor_scalar(
                                out=rt2[:, :ncols], in0=mm1p[:, :ncols],
                                scalar1=CMAGIC, scalar2=CMAGIC,
                                op0=mybir.AluOpType.add,
                                op1=mybir.AluOpType.subtract,
                            )
                            # --- MM2 (+fused transpose): data stationary ---
                            mm2p = ppool4.tile([128, 512], fp32, tag="mm2")
                            for ii, wbp in enumerate(range(w0, w1)):
                                nc.tensor.matmul(
                                    mm2p[:, ii * 128 : (ii + 1) * 128],
                                    rt2[:, ii * 128 : (ii + 1) * 128],
                                    w2_t[:, :],
                                    start=True, stop=True,
                                )
                            # --- C3: scatter back to natural (h,w) layout ---
                            c3_src = mm2p[:, :ncols].rearrange(
                                "c (wp j sk) -> c wp j sk", wp=nw, j=8, sk=16
                            )
                            if w0 == 0 or not c3_split:
                                nc.scalar.copy(yt_g[:, w0:w1, :, :], c3_src)
                            else:
                                nc.vector.tensor_copy(yt_g[:, w0:w1, :, :], c3_src)
                    nc.sync.dma_start(
                        out=y_flat[:, hbp * PAIRCHUNK : (hbp + 1) * PAIRCHUNK],
                        in_=yt[:, :],
                    )
    return nc


_PROGRAM = None


def kernel(x: np.ndarray, q_table: np.ndarray) -> np.ndarray:
    global _PROGRAM, LAST_EXEC_NS, LAST_PROFILE
    from concourse.bass_utils import run_bass_kernel_spmd

    x = np.ascontiguousarray(np.asarray(x, dtype=np.float32))
    q_table = np.asarray(q_table, dtype=np.float32)
    assert x.shape == (N_FULL, C, H, W), x.shape

    G1, W2p, I32 = _build_consts(q_table)
    if _PROGRAM is None:
        nc = _build_program()
        nc.finalize()
        _PROGRAM = nc
    nc = _PROGRAM

    core_ids = list(range(N_CORES))
    in_maps = []
    for i in core_ids:
        shard = np.ascontiguousarray(x[i * N_SHARD : (i + 1) * N_SHARD])
        in_maps.append({"x": shard, "g1": G1, "w2": W2p, "id32": I32})

    res = run_bass_kernel_spmd(nc, in_maps, core_ids)
    LAST_EXEC_NS = res.exec_time_ns
    LAST_PROFILE = res.profile_json
    out = np.concatenate([np.asarray(res.results[i]["y"]) for i in core_ids], axis=0)
    return out.astype(np.float32)
